# revision 4
# baseline (speedup 1.0000x reference)
"""Causal self-attention (QKV proj + softmax(QK^T/sqrt(N)) @ V) on 8 TRN2 cores,
v2: pair-shared-HBM K/V exchange + bf16 + tighter causal skipping.

Sharding: core c = 2*b + j handles batch element b; the pair (2b, 2b+1) splits
work. K/V projection is de-duplicated: core j projects key chunks {2j, 2j+1}
(512 keys each), scatters K/V (bf16) into addr_space='Shared' pair-HBM via
indirect DMA, barriers with a tiny 2-rank AllReduce, and reads the partner's
chunks back with plain DMA. Queries: core j takes the 8 q-tiles with global
index == j (mod 2), grouped in pairs; q-pair group gi attends key chunks
0..gi (counts [4,8,12,16] k-tiles), with on-device causal masks (from shipped
position vectors) on the diagonal chunk. All matmuls bf16 (fp32 PSUM), which
also enables fast-weight-load. Softmax runs without max subtraction; the
denominator comes from a ones-vector matmul; P^T is directly the PV lhsT.
"""

import math
from contextlib import ExitStack

import numpy as np

import concourse.bass as bass
import concourse.mybir as mybir
import concourse.tile as tile
from concourse.bass_utils import run_bass_kernel_spmd
from concourse.tile_rust import add_dep_helper

P = 128
CH = 512   # keys per chunk (and max fp32 moving free dim)
NCHUNK = 4
# q-tile assignment per pair parity: group gi = (qtiles[2gi], qtiles[2gi+1])
# attends k-tiles 0..4(gi+1); this split balances the causal padding waste
# (needs [4,7,12,15] vs [3,8,11,16] against static counts [4,8,12,16]).
QTILES = [[0, 3, 5, 6, 8, 11, 13, 14], [1, 2, 4, 7, 9, 10, 12, 15]]


def _fix_matmul_waits(nc):
    """Walrus codegen has a small per-instruction sync-wait slot budget. Move
    extra waits onto NoOps inserted just before the instruction on the same
    engine — per-engine program order (and thus semantics) is unchanged."""
    import concourse.mybir as mybir
    skip = (mybir.InstEventSemaphore, mybir.InstNoOp,
            mybir.InstUnconditionalBranch, mybir.InstCall)
    for func in nc.m.functions:
        for bb in func.blocks:
            il = bb.instructions
            new = []
            changed = False
            for inst in il:
                si = getattr(inst, "sync_info", None)
                if (si and si.on_wait and len(si.on_wait) > 1
                        and not isinstance(inst, skip)):
                    waits = list(si.on_wait)
                    for wi, w in enumerate(waits[:-1]):
                        nop = mybir.InstNoOp(
                            name=f"{inst.name}-wfix{wi}", engine=inst.engine,
                            sync_info=mybir.SyncInfo(on_wait=[w], on_update=[]),
                            text_hint="waitfix")
                        new.append(nop)
                    inst.sync_info = mybir.SyncInfo(
                        on_wait=[waits[-1]], on_update=list(si.on_update or []))
                    changed = True
                new.append(inst)
            if changed:
                bb.instructions = new


def build(N=2048, D=1024, fix_waits=True, **bass_kwargs):
    NT = N // P           # 16 key tiles
    DN = D // P           # 8 contraction / feature tiles
    QL = 8                # local q-tiles per core
    QTOT = QL * P         # 1024 query rows per core
    NG = 4                # q-pair groups
    GW = 2 * P            # 256 query columns per group
    SCALE = 1.0 / math.sqrt(N)
    BF = mybir.dt.bfloat16
    F32 = mybir.dt.float32
    I32 = mybir.dt.int32
    AF = mybir.ActivationFunctionType
    OP = mybir.AluOpType

    nc = bass.Bass(num_devices=8, **bass_kwargs)
    anchors = []   # first K-proj matmul per own chunk; DMA stage gates

    def _after(dma_bi, anchor_idx):
        if anchors and anchor_idx < len(anchors):
            add_dep_helper(dma_bi.ins, anchors[anchor_idx].ins, sync=True,
                           reason="dma staging")
        return dma_bi

    # per-core inputs (host-prepped, bf16 where matmul operands)
    ctx_ownT = nc.declare_dram_parameter("ctx_ownT", [D, 2 * CH], BF, isOutput=False)
    ctx_qT = nc.declare_dram_parameter("ctx_qT", [D, QTOT], BF, isOutput=False)
    w_qkv = nc.declare_dram_parameter("w_qkv", [D, 3 * D], BF, isOutput=False)
    qpos = nc.declare_dram_parameter("qpos", [P, QTOT], F32, isOutput=False)
    kpos = nc.declare_dram_parameter("kpos", [P, NT], F32, isOutput=False)
    bqT = nc.declare_dram_parameter("bqT", [P, DN], F32, isOutput=False)
    bkT = nc.declare_dram_parameter("bkT", [P, DN], F32, isOutput=False)
    bvb = nc.declare_dram_parameter("bvb", [P, D], F32, isOutput=False)
    onesd = nc.declare_dram_parameter("onesd", [P, 8], BF, isOutput=False)
    pidx = nc.declare_dram_parameter("pidx", [P, 1], I32, isOutput=False)
    out_ext = nc.declare_dram_parameter("out", [QTOT, D], F32, isOutput=True)

    with ExitStack() as ctx:
        tc = ctx.enter_context(tile.TileContext(nc))
        const = ctx.enter_context(tc.tile_pool(name="const", bufs=1))
        persist = ctx.enter_context(tc.tile_pool(name="persist", bufs=1))
        dram = ctx.enter_context(tc.tile_pool(name="dram", bufs=1, space="DRAM"))

        # pair-shared HBM staging for the K/V exchange. Row = parity*128 + p
        # (parity = writer core's j); global chunk/tile -> (parity, slot) is a
        # fixed map, so gathers are static. One scatter instruction per tensor.
        k_sh = dram.tile([2 * P, 2 * DN * CH], BF, addr_space="Shared", name="k_sh")
        v_sh = dram.tile([2 * P, 8 * D], BF, addr_space="Shared", name="v_sh")

        qpos_sb = const.tile([P, QTOT], F32)
        kpos_sb = const.tile([P, NT], F32)
        bq_sb = const.tile([P, DN], F32)
        nc.scalar.dma_start(out=bq_sb, in_=bqT[:, :])
        bk_sb = const.tile([P, DN], F32)
        nc.scalar.dma_start(out=bk_sb, in_=bkT[:, :])
        ones_sb = const.tile([P, 8], BF)
        nc.scalar.dma_start(out=ones_sb, in_=onesd[:, :])
        pidx_sb = const.tile([P, 1], I32)
        nc.scalar.dma_start(out=pidx_sb, in_=pidx[:, :])

        # K chunk tiles [feature-in-dtile partition, (dtile, key-in-chunk)];
        # V tiles [key partition, feature]. Local chunks are computed in
        # place; remote chunks are gathered from the partner after the barrier.
        kch = [persist.tile([P, DN * CH], BF, tag=f"kch{c}", name=f"kch{c}")
               for c in range(NCHUNK)]
        v_sb = [persist.tile([P, D], BF, tag=f"v{t}", name=f"v{t}") for t in range(NT)]

        barrd = ctx.enter_context(tc.tile_pool(name="barrd", bufs=1, space="DRAM"))
        barr_k_in = barrd.tile([P, 1], F32, name="barr_k_in")
        barr_k_out = barrd.tile([P, 1], F32, name="barr_k_out")
        barr_v_in = barrd.tile([P, 1], F32, name="barr_v_in")
        barr_v_out = barrd.tile([P, 1], F32, name="barr_v_out")
        nc.scalar.dma_start(out=barr_k_in, in_=bqT[:, 0:1])
        nc.scalar.dma_start(out=barr_v_in, in_=bkT[:, 0:1])

        # ---------------- K/V projection of OWN chunks ----------------
        # Order: K(both chunks) -> scatter+barrier-K, V(both) -> scatter+
        # barrier-V, then Q proj — each exchange hides behind >=40us of
        # tensor work. Gathers fill the globally-indexed kch/v_sb tiles.
        with tc.tile_pool(name="wkv", bufs=1) as wkv, \
             tc.tile_pool(name="ctxp", bufs=1) as ctxp, \
             tc.tile_pool(name="kvall", bufs=1) as kvallp, \
             tc.tile_pool(name="pp", bufs=6, space="PSUM") as pp:
            wk_sb = [wkv.tile([P, D], BF, tag=f"wk{d}", name=f"wk{d}") for d in range(DN)]
            wv_sb = [wkv.tile([P, D], BF, tag=f"wv{d}", name=f"wv{d}") for d in range(DN)]
            bv_sb = wkv.tile([P, D], F32, name="bv_sb")
            # startup-critical DMA stages (parallel within a stage, each stage
            # gated on the previous stage's last transfer):
            # A: W_k first halves + own ctx chunk 0 (the e<4 K matmuls' inputs)
            # B: W_k second halves + ctx chunk 1;  C: W_v;  D: bias
            wk_last = None
            for d in range(DN):
                wk_last = nc.sync.dma_start(out=wk_sb[d][:, 0:CH],
                                            in_=w_qkv[d * P:(d + 1) * P, D:D + CH])
            cts = [[None] * DN for _ in range(2)]
            ctx0_last = None
            for d in range(DN):
                ct = ctxp.tile([P, CH], BF, tag=f"ct0_{d}", name=f"ct0_{d}")
                ctx0_last = nc.scalar.dma_start(out=ct, in_=ctx_ownT[d * P:(d + 1) * P, 0:CH])
                cts[0][d] = ct
            stageB = []
            for d in range(DN):
                wk2 = nc.sync.dma_start(out=wk_sb[d][:, CH:D],
                                        in_=w_qkv[d * P:(d + 1) * P, D + CH:2 * D])
                add_dep_helper(wk2.ins, ctx0_last.ins, sync=True, reason="dma staging")
                stageB.append(wk2)
            for d in range(DN):
                ct = ctxp.tile([P, CH], BF, tag=f"ct1_{d}", name=f"ct1_{d}")
                c1d = nc.scalar.dma_start(out=ct, in_=ctx_ownT[d * P:(d + 1) * P, CH:2 * CH])
                add_dep_helper(c1d.ins, wk_last.ins, sync=True, reason="dma staging")
                stageB.append(c1d)
                cts[1][d] = ct
            for d in range(DN):
                wvd = nc.sync.dma_start(out=wv_sb[d], in_=w_qkv[d * P:(d + 1) * P, 2 * D:3 * D])
                add_dep_helper(wvd.ins, stageB[-1].ins, sync=True, reason="dma staging")
            bvd = nc.sync.dma_start(out=bv_sb, in_=bvb[:, :])
            add_dep_helper(bvd.ins, wvd.ins, sync=True, reason="dma staging")
            kall = kvallp.tile([P, 2 * DN * CH], BF, name="kall")
            vall = kvallp.tile([P, 8 * D], BF, name="vall")

            for oc in range(2):
                for e in range(DN):
                    psk = pp.tile([P, CH], F32, tag="pp", name="psk")
                    for d in range(DN):
                        mm = nc.tensor.matmul(psk, lhsT=wk_sb[d][:, e * P:(e + 1) * P],
                                              rhs=cts[oc][d], start=(d == 0), stop=(d == DN - 1))
                        if e == 0 and d == 0:
                            anchors.append(mm)
                    nc.scalar.activation(kall[:, (oc * DN + e) * CH:(oc * DN + e + 1) * CH],
                                         psk, AF.Identity, bias=bk_sb[:, e:e + 1], scale=1.0)
            ksc = nc.gpsimd.indirect_dma_start(
                out=k_sh[:, :],
                out_offset=bass.IndirectOffsetOnAxis(ap=pidx_sb[:, 0:1], axis=0),
                in_=kall[:, :],
                in_offset=None)
            cc1 = nc.gpsimd.collective_compute(
                "AllReduce", OP.add,
                replica_groups=[[0, 1], [2, 3], [4, 5], [6, 7]],
                ins=[barr_k_in.opt()], outs=[barr_k_out.opt()])
            add_dep_helper(cc1.ins, ksc.ins, sync=True, reason="K barrier after scatter")

            for oc in range(2):
                for nt_loc in range(CH // P):
                    vbase = (oc * 4 + nt_loc) * D
                    for ei, eoff in enumerate(range(0, D, CH)):
                        psv = pp.tile([P, CH], F32, tag="pp", name="psv")
                        for d in range(DN):
                            nc.tensor.matmul(psv,
                                             lhsT=cts[oc][d][:, nt_loc * P:(nt_loc + 1) * P],
                                             rhs=wv_sb[d][:, eoff:eoff + CH],
                                             start=(d == 0), stop=(d == DN - 1))
                        nc.vector.tensor_tensor(vall[:, vbase + eoff:vbase + eoff + CH], psv,
                                                bv_sb[:, eoff:eoff + CH], OP.add)
            vsc = nc.gpsimd.indirect_dma_start(
                out=v_sh[:, :],
                out_offset=bass.IndirectOffsetOnAxis(ap=pidx_sb[:, 0:1], axis=0),
                in_=vall[:, :],
                in_offset=None)
            cc2 = nc.gpsimd.collective_compute(
                "AllReduce", OP.add,
                replica_groups=[[0, 1], [2, 3], [4, 5], [6, 7]],
                ins=[barr_v_in.opt()], outs=[barr_v_out.opt()])
            add_dep_helper(cc2.ins, vsc.ins, sync=True, reason="V barrier after scatter")

        # ---------------- barrier + Q projection + gather ----------------
        with tc.tile_pool(name="wq", bufs=1) as wqp, \
             tc.tile_pool(name="ctxq", bufs=1) as ctxq, \
             tc.tile_pool(name="qtb", bufs=1) as qtb, \
             tc.tile_pool(name="att_m", bufs=3) as mpool, \
             tc.tile_pool(name="att_o", bufs=3) as opool, \
             tc.tile_pool(name="epool", bufs=1) as epool, \
             tc.tile_pool(name="ps_s", bufs=2, space="PSUM") as ps_s, \
             tc.tile_pool(name="ps_pv", bufs=4, space="PSUM") as ps_pv, \
             tc.tile_pool(name="ps_den", bufs=2, space="PSUM") as ps_den:

            # Q projection (fills the barrier window on the tensor engine)
            wq_sb = [wqp.tile([P, D], BF, tag=f"wq{d}", name=f"wq{d}") for d in range(DN)]
            for d in range(DN):
                _after(nc.sync.dma_start(out=wq_sb[d], in_=w_qkv[d * P:(d + 1) * P, 0:D]), 0)
            cqs = []
            for d in range(DN):
                cq = ctxq.tile([P, QTOT], BF, tag=f"cq{d}", name=f"cq{d}")
                _after(nc.sync.dma_start(out=cq, in_=ctx_qT[d * P:(d + 1) * P, :]), 1)
                cqs.append(cq)
            _after(nc.sync.dma_start(out=qpos_sb, in_=qpos[:, :]), 1)
            _after(nc.sync.dma_start(out=kpos_sb, in_=kpos[:, :]), 1)
            qT_sb = [qtb.tile([P, QTOT], BF, tag=f"qtb{e}", name=f"qtb{e}") for e in range(DN)]
            for e in range(DN):
                for h in range(2):
                    psq = ps_s.tile([P, CH], F32, tag="s", name="psq")
                    for d in range(DN):
                        nc.tensor.matmul(psq, lhsT=wq_sb[d][:, e * P:(e + 1) * P],
                                         rhs=cqs[d][:, h * CH:(h + 1) * CH],
                                         start=(d == 0), stop=(d == DN - 1))
                    nc.scalar.activation(qT_sb[e][:, h * CH:(h + 1) * CH], psq,
                                         AF.Identity, bias=bq_sb[:, e:e + 1], scale=1.0)

            # gather ALL chunks from shared staging (remote halves + own, cheap)
            for c in range(NCHUNK):
                jc, oc = divmod(c, 2)
                g = nc.sync.dma_start(
                    out=kch[c],
                    in_=k_sh[jc * P:(jc + 1) * P, oc * DN * CH:(oc + 1) * DN * CH])
                add_dep_helper(g.ins, cc1.ins, sync=True, reason="gather after K barrier")
            for t in range(NT):
                jt, st = divmod(t, 8)
                g = nc.sync.dma_start(
                    out=v_sb[t], in_=v_sh[jt * P:(jt + 1) * P, st * D:(st + 1) * D])
                add_dep_helper(g.ins, cc2.ins, sync=True, reason="gather after V barrier")

            # ---------------- attention ----------------
            # e[(t, gi)]: exp-scores for k-tile t vs q-group gi (256 q cols)
            e_sb = {}
            for gi in range(NG):
                for t in range(4 * gi + 4):
                    e_sb[(t, gi)] = epool.tile([P, GW], BF, tag=f"e{t}_{gi}",
                                               name=f"e{t}_{gi}")
            for c in range(NCHUNK):
                for gi in range(c, NG):
                    gcol = gi * GW
                    for tl in range(4):
                        t = 4 * c + tl
                        pss = ps_s.tile([P, GW], F32, tag="s", name="pss")
                        for d in range(DN):
                            nc.tensor.matmul(
                                pss, lhsT=kch[c][:, d * CH + tl * P:d * CH + (tl + 1) * P],
                                rhs=qT_sb[d][:, gcol:gcol + GW],
                                start=(d == 0), stop=(d == DN - 1))
                        et = e_sb[(t, gi)]
                        nc.scalar.activation(et, pss, AF.Exp, scale=SCALE)
                        if gi == c:  # diagonal chunk: causal mask
                            m = mpool.tile([P, GW], F32, tag="m", name="m")
                            nc.vector.tensor_scalar(m, qpos_sb[:, gcol:gcol + GW],
                                                    kpos_sb[:, t:t + 1], None, OP.is_ge)
                            nc.vector.tensor_tensor(et, et, m, OP.mult)
                # PV + denominator for q-group c (all its chunks are now done)
                KT = 4 * c + 4
                for r in range(2):
                    pso = [ps_pv.tile([P, CH], F32, tag="pv", name="pso")
                           for _ in range(D // CH)]
                    psd = ps_den.tile([P, 8], F32, tag="den", name="psd")
                    for t in range(KT):
                        lhsT = e_sb[(t, c)][:, r * P:(r + 1) * P]
                        for ei in range(D // CH):
                            nc.tensor.matmul(pso[ei], lhsT=lhsT,
                                             rhs=v_sb[t][:, ei * CH:(ei + 1) * CH],
                                             start=(t == 0), stop=(t == KT - 1))
                        nc.tensor.matmul(psd, lhsT=lhsT, rhs=ones_sb,
                                         start=(t == 0), stop=(t == KT - 1))
                    rec = mpool.tile([P, 1], F32, tag="rec", name="rec")
                    nc.vector.reciprocal(rec, psd[:, 0:1])
                    ql = 2 * c + r
                    for ei in range(D // CH):
                        ot = opool.tile([P, CH], F32, tag="o", name="ot")
                        nc.vector.tensor_scalar_mul(ot, pso[ei], rec)
                        nc.scalar.dma_start(
                            out=out_ext[ql * P:(ql + 1) * P, ei * CH:(ei + 1) * CH],
                            in_=ot)
    if fix_waits:
        _fix_matmul_waits(nc)
    return nc


def make_in_maps(context, W_qkv, b_qkv, n_cores=8):
    import ml_dtypes
    BF = ml_dtypes.bfloat16
    context = np.asarray(context, np.float32)
    W_qkv = np.ascontiguousarray(np.asarray(W_qkv, np.float32).astype(BF))
    b_qkv = np.asarray(b_qkv, np.float32)
    B, N, D = context.shape
    NT = N // P
    DN = D // P
    kpos = np.ascontiguousarray(
        (np.arange(NT)[None, :] * P + np.arange(P)[:, None]).astype(np.float32))
    bq = np.ascontiguousarray(b_qkv[0:D].reshape(DN, P).T)
    bk = np.ascontiguousarray(b_qkv[D:2 * D].reshape(DN, P).T)
    bv = np.ascontiguousarray(np.broadcast_to(b_qkv[2 * D:3 * D], (P, D)))
    onesd = np.ones((P, 8), BF)
    parr = np.arange(P, dtype=np.int32)
    in_maps = []
    for c in range(n_cores):
        b, j = divmod(c, 2)
        ctx_b = context[b]
        own = slice(1024 * j, 1024 * j + 1024)
        ctx_ownT = np.ascontiguousarray(ctx_b[own].T.astype(BF))
        qtiles = QTILES[j]
        ctx_q = np.concatenate([ctx_b[g * P:(g + 1) * P] for g in qtiles], axis=0)
        ctx_qT = np.ascontiguousarray(ctx_q.T.astype(BF))
        qpos_row = np.concatenate(
            [np.arange(g * P, (g + 1) * P) for g in qtiles]).astype(np.float32)
        qpos_b = np.ascontiguousarray(np.broadcast_to(qpos_row, (P, 8 * P)))
        pidx_a = (j * P + parr).reshape(P, 1)
        in_maps.append({
            "ctx_ownT": ctx_ownT, "ctx_qT": ctx_qT, "w_qkv": W_qkv,
            "qpos": qpos_b, "kpos": kpos, "bqT": bq, "bkT": bk, "bvb": bv,
            "onesd": onesd,
            "pidx": np.ascontiguousarray(pidx_a.astype(np.int32)),
        })
    return in_maps


def assemble(results, B, N, D):
    out = np.zeros((B, N, D), np.float32)
    for c, res in enumerate(results):
        b, j = divmod(c, 2)
        o = np.asarray(res["out"], np.float32)
        for l, g in enumerate(QTILES[j]):
            out[b, g * P:(g + 1) * P] = o[l * P:(l + 1) * P]
    return out


def run(inputs, trace=False, **spmd_kwargs):
    context = np.asarray(inputs["context"])
    B, N, D = context.shape
    nc = build(N, D)
    in_maps = make_in_maps(context, inputs["W_qkv"], inputs["b_qkv"], n_cores=8)
    if trace and "trace_cores" not in spmd_kwargs:
        # NRT profiling must cover every core participating in the collective
        # barriers; profiling a subset deadlocks the rendezvous.
        spmd_kwargs["trace_cores"] = list(range(8))
    res = run_bass_kernel_spmd(nc, in_maps, core_ids=list(range(8)), trace=trace, **spmd_kwargs)
    out = assemble(res.results, B, N, D)
    return out, res


def kernel(context, W_qkv, b_qkv):
    out, _ = run({"context": context, "W_qkv": W_qkv, "b_qkv": b_qkv})
    return out
